# revision 24
# baseline (speedup 1.0000x reference)
"""DGCNN classification forward pass on 8 Trainium2 NeuronCores.

Data-parallel over batch: B=16 point clouds, 2 per core. All tensor compute
runs on device; the host only shards inputs, folds BN params into weights
(constant prep), and concatenates outputs.

Per-cloud edge conv (C -> O), exploiting that BN scale>0 and LeakyReLU are
monotonic so max over neighbors commutes with the per-channel affine:
  out[o,n] = lrelu( max_k A[idx[n,k], o] + Bc[o,n] )
  A  = X^T (s*W_nbr)^T            (N, O)  -- "neighbor" projection
  Bc = (s*(W_ctr - W_nbr)) X + c  (O, N)  -- "center" part + folded BN bias

Steps:
  1. pd = 2 X^T X - d_i - d_j via PE matmul with augmented rows
     (lhsT = [2X; d; 1], rhs = [X; -1; -d]); output PSUM (128, 1024)/chunk.
  2. top-20 of each pd row: column index packed into the low 10 mantissa
     bits (one scalar_tensor_tensor AND/OR from PSUM), then top-8 per
     64-wide segment via vector.max, and top-24 of the 128 candidates via
     max/match_replace rounds. Low 10 bits of the winners = neighbor ids.
  3. dma_gather (SBUF-source fp16, transpose mode) gathers neighbor rows of
     A; output lands as (O on partitions, gather-list position on free).
  4. vector.tensor_reduce max over k -> M (O, N); out = lrelu(M + Bc).
Then conv5 (512->1024 pointwise) + global max/mean pool + 3 FC layers.
"""

import os
import numpy as np

B, C0, N = 16, 3, 1024
KNN = 20
EPS = 1e-5
SLOPE = 0.2
NCORES = 8
NCL = B // NCORES          # clouds per core
EMB = 1024

CONVS = [(3, 64), (64, 64), (64, 128), (128, 256)]   # (C_in, O)

_COMPILED = {}


def _fold_bn(bnp):
    g, b, m, v = [np.asarray(t, np.float64) for t in bnp]
    s = g / np.sqrt(v + EPS)
    c = b - s * m
    return s.astype(np.float32), c.astype(np.float32)


def prep_params(params):
    p = params
    out = {}
    for i, (ci, oi) in enumerate(CONVS, start=1):
        W = np.asarray(p[f'W{i}'], np.float32)
        s, c = _fold_bn(p[f'bn{i}'])
        Wn = (W[:, :ci] * s[:, None]).astype(np.float32)
        Wc = ((W[:, ci:] - W[:, :ci]) * s[:, None]).astype(np.float32)
        out[f'wa{i}'] = np.ascontiguousarray(Wn.T)               # (C, O)
        out[f'wb{i}'] = np.ascontiguousarray(Wc.T)               # (C, O)
        out[f'bias{i}'] = np.ascontiguousarray(
            c.reshape(-1, 128, 1) if oi > 128 else c.reshape(1, oi, 1))
    s5, c5 = _fold_bn(p['bn5'])
    W5 = np.asarray(p['W5'], np.float32) * s5[:, None]           # (1024, 512)
    W5T = np.ascontiguousarray(W5.T)                             # (512, 1024)
    # contraction pieces matching h = [x1(64); x2(64); x3(128); x4(256)]
    for k, (r0, r1) in enumerate([(0, 64), (64, 128), (128, 256),
                                  (256, 384), (384, 512)]):
        out[f'w5p{k}'] = np.ascontiguousarray(W5T[r0:r1])
    out['bias5'] = np.ascontiguousarray(c5.reshape(8, 128).T[:, :, None])
    s6, c6 = _fold_bn(p['bn6'])
    L1 = np.asarray(p['L1'], np.float32) * s6[:, None]           # (512, 2048)
    out['l1T'] = np.ascontiguousarray(
        L1.T.reshape(16, 128, 512).transpose(1, 0, 2))           # (128, 16, 512)
    out['bias6'] = np.ascontiguousarray(c6.reshape(4, 128).T[:, :, None])
    s7, c7 = _fold_bn(p['bn7'])
    L2 = np.asarray(p['L2'], np.float32) * s7[:, None]           # (256, 512)
    out['l2T'] = np.ascontiguousarray(
        L2.T.reshape(4, 128, 256).transpose(1, 0, 2))            # (128, 4, 256)
    out['bias7'] = np.ascontiguousarray(
        (s7 * np.asarray(p['b2'], np.float32) + c7)
        .reshape(2, 128).T[:, :, None])                          # (128, 2, 1)
    L3 = np.asarray(p['L3'], np.float32)                         # (40, 256)
    out['l3T'] = np.ascontiguousarray(
        L3.T.reshape(2, 128, 40).transpose(1, 0, 2))             # (128, 2, 40)
    out['bias8'] = np.asarray(p['b3'], np.float32)[:, None]      # (40, 1)
    return out


def make_in_map(x_shard, pp):
    m = {'x0': np.ascontiguousarray(x_shard, dtype=np.float32),
         'iota': np.broadcast_to(np.arange(N, dtype=np.int32)[None, :],
                                 (128, N)).copy()}
    for k, v in pp.items():
        m[k] = v
    return m


def ts(i, size):
    return slice(i * size, (i + 1) * size)


def build_kernel():
    import concourse.bass as bass
    import concourse.mybir as mybir
    from concourse import bacc
    from concourse.tile import TileContext

    fp32 = mybir.dt.float32
    i32 = mybir.dt.int32

    nc = bacc.Bacc("TRN2", debug=False)

    ins = {}
    ins['x0'] = nc.dram_tensor("x0", [NCL, C0, N], fp32,
                               kind="ExternalInput").ap()
    ins['iota'] = nc.dram_tensor("iota", [128, N], i32,
                                 kind="ExternalInput").ap()
    for i, (ci, oi) in enumerate(CONVS, start=1):
        ins[f'wa{i}'] = nc.dram_tensor(f"wa{i}", [ci, oi], fp32,
                                       kind="ExternalInput").ap()
        ins[f'wb{i}'] = nc.dram_tensor(f"wb{i}", [ci, oi], fp32,
                                       kind="ExternalInput").ap()
        ins[f'bias{i}'] = nc.dram_tensor(
            f"bias{i}", [max(1, oi // 128), min(oi, 128), 1], fp32,
            kind="ExternalInput").ap()
    for k, kw in enumerate([64, 64, 128, 128, 128]):
        ins[f'w5p{k}'] = nc.dram_tensor(f"w5p{k}", [kw, EMB], fp32,
                                        kind="ExternalInput").ap()
    ins['bias5'] = nc.dram_tensor("bias5", [128, 8, 1], fp32,
                                  kind="ExternalInput").ap()
    ins['l1T'] = nc.dram_tensor("l1T", [128, 16, 512], fp32,
                                kind="ExternalInput").ap()
    ins['bias6'] = nc.dram_tensor("bias6", [128, 4, 1], fp32,
                                  kind="ExternalInput").ap()
    ins['l2T'] = nc.dram_tensor("l2T", [128, 4, 256], fp32,
                                kind="ExternalInput").ap()
    ins['bias7'] = nc.dram_tensor("bias7", [128, 2, 1], fp32,
                                  kind="ExternalInput").ap()
    ins['l3T'] = nc.dram_tensor("l3T", [128, 2, 40], fp32,
                                kind="ExternalInput").ap()
    ins['bias8'] = nc.dram_tensor("bias8", [40, 1], fp32,
                                  kind="ExternalInput").ap()
    z_out = nc.dram_tensor("z", [NCL, 40], fp32, kind="ExternalOutput").ap()

    with TileContext(nc) as tc:
        _emit(tc, nc, mybir, bass, ins, z_out)
    nc.compile()
    return nc


def _emit(tc, nc, mybir, bass, ins, z_out):
    from contextlib import ExitStack
    fp32 = mybir.dt.float32
    fp32r = mybir.dt.float32r
    fp16 = mybir.dt.float16
    i32 = mybir.dt.int32
    i16 = mybir.dt.int16
    Alu = mybir.AluOpType
    Act = mybir.ActivationFunctionType
    MM_DT = os.environ.get("DGCNN_MM_DT", "fp32")

    def mmdt(ap):
        return ap.bitcast(fp32r) if MM_DT == "fp32r" else ap

    with ExitStack() as top:
        persist = top.enter_context(tc.tile_pool(name="persist", bufs=1))

        # persistent per-cloud features: x0 and conv outputs (chunk-in-free
        # for O=256). All fp32, layout (min(C,128), n_chunks*N).
        feats = {}
        for cl in range(NCL):
            feats[(cl, 0)] = persist.tile([C0, N], fp32, tag=f"x0_{cl}", name=f"x0_{cl}")
            for i, (ci, oi) in enumerate(CONVS, start=1):
                feats[(cl, i)] = persist.tile(
                    [min(oi, 128), max(1, oi // 128) * N], fp32,
                    tag=f"x{i}_{cl}", name=f"x{i}_{cl}")
        # global pool accumulators: [128, emb_chunk 8, {max,sum} 2, cl]
        gpool = persist.tile([128, 8, 2, NCL], fp32)

        # ---------------- phase 1: edge convs ----------------
        with ExitStack() as ph1:
            const = ph1.enter_context(tc.tile_pool(name="const1", bufs=1))
            work = ph1.enter_context(tc.tile_pool(name="work", bufs=2))
            work1 = ph1.enter_context(tc.tile_pool(name="work1", bufs=1))
            pdpool = ph1.enter_context(tc.tile_pool(name="pdpool", bufs=3))
            gath = ph1.enter_context(tc.tile_pool(name="gath", bufs=1))
            dram = ph1.enter_context(
                tc.tile_pool(name="dram", bufs=2, space="DRAM"))
            psPD = ph1.enter_context(
                tc.tile_pool(name="psPD", bufs=2, space="PSUM"))
            psB = ph1.enter_context(
                tc.tile_pool(name="psB", bufs=1, space="PSUM"))
            psA = ph1.enter_context(
                tc.tile_pool(name="psA", bufs=2, space="PSUM"))

            from concourse import library_config
            nc.gpsimd.load_library(library_config.mlp)
            iota = const.tile([128, N], i32)
            nc.sync.dma_start(out=iota, in_=ins['iota'])
            ones_col = const.tile([128, 1], fp32)
            nc.vector.memset(ones_col, 1.0)
            neg_ones = const.tile([1, N], fp32)
            nc.vector.memset(neg_ones, -1.0)
            mask_hi = const.tile([128, 1], i32)
            nc.vector.memset(mask_hi, -1024)
            mask_lo = const.tile([128, 1], i32)
            nc.vector.memset(mask_lo, 1023)
            wa_sb, wb_sb, bias_sb = {}, {}, {}
            for i, (ci, oi) in enumerate(CONVS, start=1):
                wa_sb[i] = const.tile([ci, oi], fp32, tag=f"wa{i}", name=f"wa{i}_sb")
                nc.sync.dma_start(out=wa_sb[i], in_=ins[f'wa{i}'])
                wb_sb[i] = const.tile([ci, oi], fp32, tag=f"wb{i}", name=f"wb{i}_sb")
                nc.sync.dma_start(out=wb_sb[i], in_=ins[f'wb{i}'])
                bias_sb[i] = const.tile([min(oi, 128), max(1, oi // 128), 1],
                                        fp32, tag=f"bias{i}", name=f"bias{i}_sb")
                nc.sync.dma_start(
                    out=bias_sb[i],
                    in_=ins[f'bias{i}'].rearrange("a b c -> b a c"))

            for cl in range(NCL):
                nc.sync.dma_start(out=feats[(cl, 0)], in_=ins['x0'][cl])

            P = dict(const=const, work=work, work1=work1, pdpool=pdpool,
                     gath=gath, dram=dram, psPD=psPD, psB=psB, psA=psA,
                     neg_ones=neg_ones, mask_hi=mask_hi, mask_lo=mask_lo)
            for cl in range(NCL):
                for i, (ci, oi) in enumerate(CONVS, start=1):
                    _edge_conv(tc, nc, mybir, P,
                               xin=feats[(cl, i - 1)],
                               xout=feats[(cl, i)],
                               ci=ci, oi=oi, wa=wa_sb[i], wb=wb_sb[i],
                               bias=bias_sb[i], iota=iota,
                               ones_col=ones_col, mmdt=mmdt)

        # ---------------- phase 2: conv5 + pool + FC tail ----------------
        with ExitStack() as ph2:
            const2 = ph2.enter_context(tc.tile_pool(name="const2", bufs=1))
            work2 = ph2.enter_context(tc.tile_pool(name="work2", bufs=2))
            psY = ph2.enter_context(
                tc.tile_pool(name="psY", bufs=2, space="PSUM"))
            psZ = ph2.enter_context(
                tc.tile_pool(name="psZ", bufs=2, space="PSUM"))

            w5p = []
            for k, kw in enumerate([64, 64, 128, 128, 128]):
                t = const2.tile([kw, EMB], fp32, tag=f"w5p{k}", name=f"w5p{k}_sb")
                nc.sync.dma_start(out=t, in_=ins[f'w5p{k}'])
                w5p.append(t)
            bias5 = const2.tile([128, 8, 1], fp32)
            nc.sync.dma_start(out=bias5, in_=ins['bias5'])
            l1 = const2.tile([128, 16, 512], fp32)
            nc.sync.dma_start(out=l1, in_=ins['l1T'])
            bias6 = const2.tile([128, 4, 1], fp32)
            nc.sync.dma_start(out=bias6, in_=ins['bias6'])
            l2 = const2.tile([128, 4, 256], fp32)
            nc.sync.dma_start(out=l2, in_=ins['l2T'])
            bias7 = const2.tile([128, 2, 1], fp32)
            nc.sync.dma_start(out=bias7, in_=ins['bias7'])
            l3 = const2.tile([128, 2, 40], fp32)
            nc.sync.dma_start(out=l3, in_=ins['l3T'])
            bias8 = const2.tile([40, 1], fp32)
            nc.sync.dma_start(out=bias8, in_=ins['bias8'])

            for cl in range(NCL):
                # h = [x1; x2; x3; x4]: contraction pieces (tile, chunk, rows)
                x1, x2, x3, x4 = (feats[(cl, i)] for i in (1, 2, 3, 4))
                pieces = [(x1, 0, 64), (x2, 0, 64), (x3, 0, 128),
                          (x4, 0, 128), (x4, 1, 128)]
                for ec in range(8):
                    yps = psY.tile([128, N], fp32, tag="yps")
                    for nb in range(2):
                        for ki, (xt, xc, kw) in enumerate(pieces):
                            nc.tensor.matmul(
                                yps[:, ts(nb, 512)],
                                lhsT=mmdt(w5p[ki][:, ts(ec, 128)]),
                                rhs=mmdt(xt[:kw,
                                            xc * N + nb * 512:
                                            xc * N + (nb + 1) * 512]),
                                start=(ki == 0), stop=(ki == len(pieces) - 1))
                    ysb = work2.tile([128, N], fp32, tag="ysb")
                    s_lin = work2.tile([128, 1], fp32, tag="s_lin")
                    nc.scalar.activation(out=ysb, in_=yps, func=Act.Identity,
                                         bias=bias5[:, ec, :],
                                         accum_out=s_lin)
                    yabs = work2.tile([128, N], fp32, tag="yabs")
                    s_abs = work2.tile([128, 1], fp32, tag="s_abs")
                    nc.scalar.activation(out=yabs, in_=yps, func=Act.Abs,
                                         bias=bias5[:, ec, :],
                                         accum_out=s_abs)
                    # sum(lrelu(y)) = 0.6*sum(y) + 0.4*sum(|y|)
                    nc.vector.scalar_tensor_tensor(
                        out=gpool[:, ec, 1, cl:cl + 1], in0=s_abs,
                        scalar=(1.0 - SLOPE) / (1.0 + SLOPE), in1=s_lin,
                        op0=Alu.mult, op1=Alu.add)
                    nc.vector.tensor_scalar_mul(
                        gpool[:, ec, 1, cl:cl + 1],
                        gpool[:, ec, 1, cl:cl + 1], (1.0 + SLOPE) / 2.0)
                    # max(lrelu(y)) = lrelu(max(y))
                    pmax = work2.tile([128, 1], fp32, tag="pmax")
                    nc.vector.tensor_reduce(
                        out=pmax, in_=ysb,
                        axis=mybir.AxisListType.X, op=Alu.max)
                    nc.vector.scalar_tensor_tensor(
                        out=gpool[:, ec, 0, cl:cl + 1], in0=pmax,
                        scalar=SLOPE, in1=pmax, op0=Alu.mult, op1=Alu.max)

            # mean = sum / N
            nc.vector.tensor_scalar_mul(gpool[:, :, 1, :],
                                        gpool[:, :, 1, :], 1.0 / N)

            # FC tail. g chunks: gc in [0,8) = max emb chunks, [8,16) = mean
            z1 = work2.tile([128, 4, NCL], fp32, tag="z1")
            for zc in range(4):
                zps = psZ.tile([128, NCL], fp32, tag="zps")
                for gc in range(16):
                    which, ec = gc // 8, gc % 8
                    nc.tensor.matmul(
                        zps, lhsT=mmdt(l1[:, gc, ts(zc, 128)]),
                        rhs=mmdt(gpool[:, ec, which, :]),
                        start=(gc == 0), stop=(gc == 15))
                nc.scalar.activation(out=z1[:, zc, :], in_=zps,
                                     func=Act.Identity, bias=bias6[:, zc, :])
                nc.vector.scalar_tensor_tensor(
                    out=z1[:, zc, :], in0=z1[:, zc, :], scalar=SLOPE,
                    in1=z1[:, zc, :], op0=Alu.mult, op1=Alu.max)
            z2 = work2.tile([128, 2, NCL], fp32, tag="z2")
            for zc in range(2):
                zps = psZ.tile([128, NCL], fp32, tag="zps")
                for gc in range(4):
                    nc.tensor.matmul(
                        zps, lhsT=mmdt(l2[:, gc, ts(zc, 128)]),
                        rhs=mmdt(z1[:, gc, :]),
                        start=(gc == 0), stop=(gc == 3))
                nc.scalar.activation(out=z2[:, zc, :], in_=zps,
                                     func=Act.Identity, bias=bias7[:, zc, :])
                nc.vector.scalar_tensor_tensor(
                    out=z2[:, zc, :], in0=z2[:, zc, :], scalar=SLOPE,
                    in1=z2[:, zc, :], op0=Alu.mult, op1=Alu.max)
            zps = psZ.tile([40, NCL], fp32, tag="zps40")
            for gc in range(2):
                nc.tensor.matmul(zps, lhsT=mmdt(l3[:, gc, :]),
                                 rhs=mmdt(z2[:, gc, :]),
                                 start=(gc == 0), stop=(gc == 1))
            zf = work2.tile([40, NCL], fp32, tag="zf")
            nc.scalar.activation(out=zf, in_=zps, func=Act.Identity,
                                 bias=bias8)
            nc.sync.dma_start(out=z_out.rearrange("c o -> o c"), in_=zf)


def _edge_conv(tc, nc, mybir, P, xin, xout, ci, oi, wa, wb, bias, iota,
               ones_col, mmdt):
    fp32 = mybir.dt.float32
    fp16 = mybir.dt.float16
    i32 = mybir.dt.int32
    i16 = mybir.dt.int16
    Alu = mybir.AluOpType
    Act = mybir.ActivationFunctionType

    work, work1, pdpool, gath = P['work'], P['work1'], P['pdpool'], P['gath']
    psPD, psB, psA, dram = P['psPD'], P['psB'], P['psA'], P['dram']

    o_chunks = max(1, oi // 128)

    # ---- d, augmented rows ----
    xsq = work.tile([ci, N], fp32, tag="xsq")
    nc.scalar.activation(out=xsq, in_=xin, func=Act.Square)
    dps = psB.tile([1, N], fp32, tag="bigps")
    for nb in range(2):
        nc.tensor.matmul(dps[:, ts(nb, 512)], lhsT=ones_col[:ci, :],
                         rhs=xsq[:, ts(nb, 512)], start=True, stop=True)
    lhsT2x = work.tile([ci, N], fp32, tag="lhsT2x")
    nc.vector.tensor_scalar_mul(lhsT2x, xin, 2.0)
    # aug rows: engines can only write partition 0, so build single-row
    # tiles and DMA them into partition 1 of the aug tiles.
    d_row = work.tile([1, N], fp32, tag="d_row")
    nc.scalar.activation(out=d_row, in_=dps, func=Act.Copy)
    aug_l = work.tile([2, N], fp32, tag="aug_l")      # [ones; d]
    nc.vector.memset(aug_l[0:1, :], 1.0)
    nc.sync.dma_start(out=aug_l[1:2, :], in_=d_row)
    aug_r = work.tile([2, N], fp32, tag="aug_r")      # [-d; -ones]
    nc.scalar.activation(out=aug_r[0:1, :], in_=dps, func=Act.Copy,
                         scale=-1.0)
    nc.sync.dma_start(out=aug_r[1:2, :], in_=P['neg_ones'])

    # ---- A (O, N) fp32: same orientation as Bc ----
    a_sb = gath.tile([128, o_chunks * N], fp32, tag="a_sb")
    for oc in range(o_chunks):
        ow = min(oi, 128)
        for nb in range(2):
            aps = psA.tile([128, 512], fp32, tag="aps")
            nc.tensor.matmul(
                aps[:ow, :], lhsT=mmdt(wa[:, oc * 128:oc * 128 + ow]),
                rhs=mmdt(xin[:, ts(nb, 512)]), start=True, stop=True)
            nc.scalar.activation(
                out=a_sb[:ow, oc * N + nb * 512:oc * N + (nb + 1) * 512],
                in_=aps[:ow, :], func=Act.Copy)

    # ---- Bc (O, N) fp32 with folded bias ----
    bc_sb = work1.tile([128, o_chunks * N], fp32, tag="bc_sb")
    for oc in range(o_chunks):
        ow = min(oi, 128)
        bps = psB.tile([ow, N], fp32, tag="bigps")
        for nb in range(2):
            nc.tensor.matmul(
                bps[:, ts(nb, 512)],
                lhsT=mmdt(wb[:, oc * 128:oc * 128 + ow]),
                rhs=mmdt(xin[:, ts(nb, 512)]), start=True, stop=True)
        nc.scalar.activation(out=bc_sb[:ow, ts(oc, N)], in_=bps,
                             func=Act.Identity, bias=bias[:ow, oc, :])

    # ---- pd + topk ----
    idx16 = work.tile([128, 8, 24], i16, tag="idx16")
    for rc in range(8):
        pdps = psPD.tile([128, N], fp32, tag="pdps")
        for nb in range(2):
            nc.tensor.matmul(pdps[:, ts(nb, 512)],
                             lhsT=mmdt(lhsT2x[:, ts(rc, 128)]),
                             rhs=mmdt(xin[:, ts(nb, 512)]),
                             start=True, stop=False)
            nc.tensor.matmul(pdps[:, ts(nb, 512)],
                             lhsT=mmdt(aug_l[:, ts(rc, 128)]),
                             rhs=mmdt(aug_r[:, ts(nb, 512)]),
                             start=False, stop=True)
        pdsb = pdpool.tile([128, N], i32, tag="pdsb")
        nc.vector.scalar_tensor_tensor(
            out=pdsb, in0=pdps.bitcast(i32), scalar=P['mask_hi'], in1=iota,
            op0=Alu.bitwise_and, op1=Alu.bitwise_or)
        pdf = pdsb.bitcast(fp32)
        cand = pdpool.tile([128, 128], fp32, tag="cand")
        for s in range(16):
            nc.vector.max(out=cand[:, ts(s, 8)], in_=pdf[:, ts(s, 64)])
        top24 = pdpool.tile([128, 24], fp32, tag="top24")
        nc.vector.max(out=top24[:, 0:8], in_=cand)
        nc.vector.match_replace(out=cand, in_to_replace=top24[:, 0:8],
                                in_values=cand, imm_value=-1e30)
        nc.vector.max(out=top24[:, 8:16], in_=cand)
        nc.vector.match_replace(out=cand, in_to_replace=top24[:, 8:16],
                                in_values=cand, imm_value=-1e30)
        nc.vector.max(out=top24[:, 16:24], in_=cand)
        idx32 = pdpool.tile([128, 24], i32, tag="idx32")
        nc.vector.scalar_tensor_tensor(
            out=idx32, in0=top24.bitcast(i32), scalar=P['mask_lo'],
            in1=P['mask_lo'].to_broadcast([128, 24]),
            op0=Alu.bitwise_and, op1=Alu.bitwise_and)
        nc.vector.tensor_copy(
            out=idx16[:, rc, :],
            in_=idx32.bitcast(i16).rearrange("p (a b) -> p a b", b=2)[:, :, 0])

    # ---- gather + reduce + lrelu(M + Bc), per 128-row chunk ----
    # gather position within chunk rc: i = (nhi*20 + k)*16 + nlo;
    # wrap layout: p = i%16 = nlo, slot = i//16 = nhi*20 + k.
    # Full-image dram bounce: addr = nlo*1280 + (rc*8+nhi)*20 + k so the
    # per-rc wrapped image is dscratch[:, rc*160:(rc+1)*160].
    NIDX = N * KNN
    ow = min(oi, 128)
    dscratch = dram.tile([16, NIDX // 16], i16, tag="dscratch",
                         name="dscratch")
    dst = dscratch.rearrange("nlo (rc nhi k) -> nhi nlo rc k",
                             rc=8, nhi=8, k=KNN)
    nc.sync.dma_start(out=dst, in_=idx16[:, :, :KNN])
    idxw = work.tile([128, NIDX // 16], i16, tag="idxw", name="idxw")
    for grp in range(8):
        nc.sync.dma_start(out=idxw[grp * 16:(grp + 1) * 16, :], in_=dscratch)
    NR = 128 * KNN                            # 2560 idxs per row chunk
    for rc in range(8):
        for oc in range(o_chunks):
            gthr = gath.tile([128, NR], fp32, tag="gthr", name="gthr")
            cw = max(16, (ow + 15) // 16 * 16)
            nc.gpsimd.ap_gather(
                out_ap=gthr[:cw, :], in_ap=a_sb[:cw, ts(oc, N)],
                idxs_ap=idxw[:cw, ts(rc, NR // 16)],
                channels=cw,
                num_elems=N, d=1, num_idxs=NR)
            g4 = gthr[:ow, :].rearrange("p (nhi k nlo) -> p nhi nlo k",
                                        nhi=8, k=KNN, nlo=16)
            m32 = gath.tile([128, 128], fp32, tag="m32", name="m32")
            nc.vector.tensor_reduce(
                out=m32[:ow, :].rearrange("p (nhi nlo) -> p nhi nlo", nhi=8),
                in_=g4, axis=mybir.AxisListType.X, op=Alu.max)
            osl = slice(oc * N + rc * 128, oc * N + rc * 128 + 128)
            nc.vector.tensor_tensor(out=xout[:ow, osl], in0=m32[:ow, :],
                                    in1=bc_sb[:ow, osl], op=Alu.add)
            nc.vector.scalar_tensor_tensor(
                out=xout[:ow, osl], in0=xout[:ow, osl], scalar=SLOPE,
                in1=xout[:ow, osl], op0=Alu.mult, op1=Alu.max)


def kernel(x, params):
    from concourse.bass_utils import run_bass_kernel_spmd

    x = np.asarray(x, np.float32)
    pp = prep_params(params)
    if 'nc' not in _COMPILED:
        _COMPILED['nc'] = build_kernel()
    nc = _COMPILED['nc']

    in_maps = [make_in_map(x[core * NCL:(core + 1) * NCL], pp)
               for core in range(NCORES)]
    res = run_bass_kernel_spmd(nc, in_maps, core_ids=list(range(NCORES)))
    return np.concatenate([r['z'] for r in res.results],
                          axis=0).astype(np.float32)


# revision 26
# speedup vs baseline: 1.2191x; 1.2191x over previous
"""DGCNN classification forward pass on 8 Trainium2 NeuronCores.

Data-parallel over batch: B=16 point clouds, 2 per core. All tensor compute
runs on device; the host only shards inputs, folds BN params into weights
(constant prep), and concatenates outputs.

Per-cloud edge conv (C -> O), exploiting that BN scale>0 and LeakyReLU are
monotonic so max over neighbors commutes with the per-channel affine:
  out[o,n] = lrelu( max_k A[idx[n,k], o] + Bc[o,n] )
  A  = X^T (s*W_nbr)^T            (N, O)  -- "neighbor" projection
  Bc = (s*(W_ctr - W_nbr)) X + c  (O, N)  -- "center" part + folded BN bias

Steps:
  1. pd = 2 X^T X - d_i - d_j via PE matmul with augmented rows
     (lhsT = [2X; d; 1], rhs = [X; -1; -d]); output PSUM (128, 1024)/chunk.
  2. top-20 of each pd row: column index packed into the low 10 mantissa
     bits (one scalar_tensor_tensor AND/OR from PSUM), then top-8 per
     64-wide segment via vector.max, and top-24 of the 128 candidates via
     max/match_replace rounds. Low 10 bits of the winners = neighbor ids.
  3. gpsimd ap_gather on the (O, N) fp32 projection A gathers the 20
     neighbor columns per point (indices wrapped per 16 partitions).
  4. vector.tensor_reduce max over k -> M (O, N); out = lrelu(M + Bc).
Then conv5 (512->1024 pointwise) + global max/mean pool + 3 FC layers.
"""

import os
import numpy as np

B, C0, N = 16, 3, 1024
KNN = 20
EPS = 1e-5
SLOPE = 0.2
NCORES = 8
NCL = B // NCORES          # clouds per core
EMB = 1024

CONVS = [(3, 64), (64, 64), (64, 128), (128, 256)]   # (C_in, O)

_COMPILED = {}


def _fold_bn(bnp):
    g, b, m, v = [np.asarray(t, np.float64) for t in bnp]
    s = g / np.sqrt(v + EPS)
    c = b - s * m
    return s.astype(np.float32), c.astype(np.float32)


def prep_params(params):
    p = params
    out = {}
    for i, (ci, oi) in enumerate(CONVS, start=1):
        W = np.asarray(p[f'W{i}'], np.float32)
        s, c = _fold_bn(p[f'bn{i}'])
        Wn = (W[:, :ci] * s[:, None]).astype(np.float32)
        Wc = ((W[:, ci:] - W[:, :ci]) * s[:, None]).astype(np.float32)
        out[f'wa{i}'] = np.ascontiguousarray(Wn.T)               # (C, O)
        out[f'wb{i}'] = np.ascontiguousarray(Wc.T)               # (C, O)
        out[f'bias{i}'] = np.ascontiguousarray(
            c.reshape(-1, 128, 1) if oi > 128 else c.reshape(1, oi, 1))
    s5, c5 = _fold_bn(p['bn5'])
    W5 = np.asarray(p['W5'], np.float32) * s5[:, None]           # (1024, 512)
    W5T = np.ascontiguousarray(W5.T)                             # (512, 1024)
    # contraction pieces matching h = [x1(64); x2(64); x3(128); x4(256)]
    for k, (r0, r1) in enumerate([(0, 64), (64, 128), (128, 256),
                                  (256, 384), (384, 512)]):
        out[f'w5p{k}'] = np.ascontiguousarray(W5T[r0:r1])
    out['bias5'] = np.ascontiguousarray(c5.reshape(8, 128).T[:, :, None])
    s6, c6 = _fold_bn(p['bn6'])
    L1 = np.asarray(p['L1'], np.float32) * s6[:, None]           # (512, 2048)
    out['l1T'] = np.ascontiguousarray(
        L1.T.reshape(16, 128, 512).transpose(1, 0, 2))           # (128, 16, 512)
    out['bias6'] = np.ascontiguousarray(c6.reshape(4, 128).T[:, :, None])
    s7, c7 = _fold_bn(p['bn7'])
    L2 = np.asarray(p['L2'], np.float32) * s7[:, None]           # (256, 512)
    out['l2T'] = np.ascontiguousarray(
        L2.T.reshape(4, 128, 256).transpose(1, 0, 2))            # (128, 4, 256)
    out['bias7'] = np.ascontiguousarray(
        (s7 * np.asarray(p['b2'], np.float32) + c7)
        .reshape(2, 128).T[:, :, None])                          # (128, 2, 1)
    L3 = np.asarray(p['L3'], np.float32)                         # (40, 256)
    out['l3T'] = np.ascontiguousarray(
        L3.T.reshape(2, 128, 40).transpose(1, 0, 2))             # (128, 2, 40)
    out['bias8'] = np.asarray(p['b3'], np.float32)[:, None]      # (40, 1)
    return out


def make_in_map(x_shard, pp):
    m = {'x0': np.ascontiguousarray(x_shard, dtype=np.float32),
         'iota': np.broadcast_to(np.arange(N, dtype=np.int32)[None, :],
                                 (128, N)).copy()}
    for k, v in pp.items():
        m[k] = v
    return m


def ts(i, size):
    return slice(i * size, (i + 1) * size)


def build_kernel():
    import concourse.bass as bass
    import concourse.mybir as mybir
    from concourse import bacc
    from concourse.tile import TileContext

    fp32 = mybir.dt.float32
    i32 = mybir.dt.int32

    nc = bacc.Bacc("TRN2", debug=False)

    ins = {}
    ins['x0'] = nc.dram_tensor("x0", [NCL, C0, N], fp32,
                               kind="ExternalInput").ap()
    ins['iota'] = nc.dram_tensor("iota", [128, N], i32,
                                 kind="ExternalInput").ap()
    for i, (ci, oi) in enumerate(CONVS, start=1):
        ins[f'wa{i}'] = nc.dram_tensor(f"wa{i}", [ci, oi], fp32,
                                       kind="ExternalInput").ap()
        ins[f'wb{i}'] = nc.dram_tensor(f"wb{i}", [ci, oi], fp32,
                                       kind="ExternalInput").ap()
        ins[f'bias{i}'] = nc.dram_tensor(
            f"bias{i}", [max(1, oi // 128), min(oi, 128), 1], fp32,
            kind="ExternalInput").ap()
    for k, kw in enumerate([64, 64, 128, 128, 128]):
        ins[f'w5p{k}'] = nc.dram_tensor(f"w5p{k}", [kw, EMB], fp32,
                                        kind="ExternalInput").ap()
    ins['bias5'] = nc.dram_tensor("bias5", [128, 8, 1], fp32,
                                  kind="ExternalInput").ap()
    ins['l1T'] = nc.dram_tensor("l1T", [128, 16, 512], fp32,
                                kind="ExternalInput").ap()
    ins['bias6'] = nc.dram_tensor("bias6", [128, 4, 1], fp32,
                                  kind="ExternalInput").ap()
    ins['l2T'] = nc.dram_tensor("l2T", [128, 4, 256], fp32,
                                kind="ExternalInput").ap()
    ins['bias7'] = nc.dram_tensor("bias7", [128, 2, 1], fp32,
                                  kind="ExternalInput").ap()
    ins['l3T'] = nc.dram_tensor("l3T", [128, 2, 40], fp32,
                                kind="ExternalInput").ap()
    ins['bias8'] = nc.dram_tensor("bias8", [40, 1], fp32,
                                  kind="ExternalInput").ap()
    z_out = nc.dram_tensor("z", [NCL, 40], fp32, kind="ExternalOutput").ap()

    with TileContext(nc) as tc:
        _emit(tc, nc, mybir, bass, ins, z_out)
    nc.compile()
    return nc


def _emit(tc, nc, mybir, bass, ins, z_out):
    from contextlib import ExitStack
    fp32 = mybir.dt.float32
    fp32r = mybir.dt.float32r
    fp16 = mybir.dt.float16
    i32 = mybir.dt.int32
    i16 = mybir.dt.int16
    Alu = mybir.AluOpType
    Act = mybir.ActivationFunctionType
    MM_DT = os.environ.get("DGCNN_MM_DT", "fp32")

    def mmdt(ap):
        return ap.bitcast(fp32r) if MM_DT == "fp32r" else ap

    with ExitStack() as top:
        persist = top.enter_context(tc.tile_pool(name="persist", bufs=1))

        # persistent per-cloud features: x0 and conv outputs (chunk-in-free
        # for O=256). All fp32, layout (min(C,128), n_chunks*N).
        feats = {}
        for cl in range(NCL):
            feats[(cl, 0)] = persist.tile([C0, N], fp32, tag=f"x0_{cl}", name=f"x0_{cl}")
            for i, (ci, oi) in enumerate(CONVS, start=1):
                feats[(cl, i)] = persist.tile(
                    [min(oi, 128), max(1, oi // 128) * N], fp32,
                    tag=f"x{i}_{cl}", name=f"x{i}_{cl}")
        # global pool accumulators: [128, emb_chunk 8, {max,sum} 2, cl]
        gpool = persist.tile([128, 8, 2, NCL], fp32)

        # ---------------- phase 1: edge convs ----------------
        with ExitStack() as ph1:
            const = ph1.enter_context(tc.tile_pool(name="const1", bufs=1))
            work = ph1.enter_context(tc.tile_pool(name="work", bufs=2))
            work1 = ph1.enter_context(tc.tile_pool(name="work1", bufs=1))
            pdpool = ph1.enter_context(tc.tile_pool(name="pdpool", bufs=3))
            gath = ph1.enter_context(tc.tile_pool(name="gath", bufs=2))
            dram = ph1.enter_context(
                tc.tile_pool(name="dram", bufs=2, space="DRAM"))
            psPD = ph1.enter_context(
                tc.tile_pool(name="psPD", bufs=2, space="PSUM"))
            psB = ph1.enter_context(
                tc.tile_pool(name="psB", bufs=1, space="PSUM"))
            psA = ph1.enter_context(
                tc.tile_pool(name="psA", bufs=2, space="PSUM"))

            from concourse import library_config
            nc.gpsimd.load_library(library_config.mlp)
            iota = const.tile([128, N], i32)
            nc.sync.dma_start(out=iota, in_=ins['iota'])
            ones_col = const.tile([128, 1], fp32)
            nc.vector.memset(ones_col, 1.0)
            neg_ones = const.tile([1, N], fp32)
            nc.vector.memset(neg_ones, -1.0)
            mask_hi = const.tile([128, 1], i32)
            nc.vector.memset(mask_hi, -1024)
            mask_lo = const.tile([128, 1], i32)
            nc.vector.memset(mask_lo, 1023)
            wa_sb, wb_sb, bias_sb = {}, {}, {}
            for i, (ci, oi) in enumerate(CONVS, start=1):
                wa_sb[i] = const.tile([ci, oi], fp32, tag=f"wa{i}", name=f"wa{i}_sb")
                nc.sync.dma_start(out=wa_sb[i], in_=ins[f'wa{i}'])
                wb_sb[i] = const.tile([ci, oi], fp32, tag=f"wb{i}", name=f"wb{i}_sb")
                nc.sync.dma_start(out=wb_sb[i], in_=ins[f'wb{i}'])
                bias_sb[i] = const.tile([min(oi, 128), max(1, oi // 128), 1],
                                        fp32, tag=f"bias{i}", name=f"bias{i}_sb")
                nc.sync.dma_start(
                    out=bias_sb[i],
                    in_=ins[f'bias{i}'].rearrange("a b c -> b a c"))

            for cl in range(NCL):
                nc.sync.dma_start(out=feats[(cl, 0)], in_=ins['x0'][cl])

            P = dict(const=const, work=work, work1=work1, pdpool=pdpool,
                     gath=gath, dram=dram, psPD=psPD, psB=psB, psA=psA,
                     neg_ones=neg_ones, mask_hi=mask_hi, mask_lo=mask_lo)
            for cl in range(NCL):
                for i, (ci, oi) in enumerate(CONVS, start=1):
                    _edge_conv(tc, nc, mybir, P,
                               xin=feats[(cl, i - 1)],
                               xout=feats[(cl, i)],
                               ci=ci, oi=oi, wa=wa_sb[i], wb=wb_sb[i],
                               bias=bias_sb[i], iota=iota,
                               ones_col=ones_col, mmdt=mmdt)

        # ---------------- phase 2: conv5 + pool + FC tail ----------------
        with ExitStack() as ph2:
            const2 = ph2.enter_context(tc.tile_pool(name="const2", bufs=1))
            work2 = ph2.enter_context(tc.tile_pool(name="work2", bufs=2))
            psY = ph2.enter_context(
                tc.tile_pool(name="psY", bufs=2, space="PSUM"))
            psZ = ph2.enter_context(
                tc.tile_pool(name="psZ", bufs=2, space="PSUM"))

            w5p = []
            for k, kw in enumerate([64, 64, 128, 128, 128]):
                t = const2.tile([kw, EMB], fp32, tag=f"w5p{k}", name=f"w5p{k}_sb")
                nc.sync.dma_start(out=t, in_=ins[f'w5p{k}'])
                w5p.append(t)
            bias5 = const2.tile([128, 8, 1], fp32)
            nc.sync.dma_start(out=bias5, in_=ins['bias5'])
            l1 = const2.tile([128, 16, 512], fp32)
            nc.sync.dma_start(out=l1, in_=ins['l1T'])
            bias6 = const2.tile([128, 4, 1], fp32)
            nc.sync.dma_start(out=bias6, in_=ins['bias6'])
            l2 = const2.tile([128, 4, 256], fp32)
            nc.sync.dma_start(out=l2, in_=ins['l2T'])
            bias7 = const2.tile([128, 2, 1], fp32)
            nc.sync.dma_start(out=bias7, in_=ins['bias7'])
            l3 = const2.tile([128, 2, 40], fp32)
            nc.sync.dma_start(out=l3, in_=ins['l3T'])
            bias8 = const2.tile([40, 1], fp32)
            nc.sync.dma_start(out=bias8, in_=ins['bias8'])

            for cl in range(NCL):
                # h = [x1; x2; x3; x4]: contraction pieces (tile, chunk, rows)
                x1, x2, x3, x4 = (feats[(cl, i)] for i in (1, 2, 3, 4))
                pieces = [(x1, 0, 64), (x2, 0, 64), (x3, 0, 128),
                          (x4, 0, 128), (x4, 1, 128)]
                for ec in range(8):
                    yps = psY.tile([128, N], fp32, tag="yps")
                    for nb in range(2):
                        for ki, (xt, xc, kw) in enumerate(pieces):
                            nc.tensor.matmul(
                                yps[:, ts(nb, 512)],
                                lhsT=mmdt(w5p[ki][:, ts(ec, 128)]),
                                rhs=mmdt(xt[:kw,
                                            xc * N + nb * 512:
                                            xc * N + (nb + 1) * 512]),
                                start=(ki == 0), stop=(ki == len(pieces) - 1))
                    ysb = work2.tile([128, N], fp32, tag="ysb")
                    s_lin = work2.tile([128, 1], fp32, tag="s_lin")
                    nc.scalar.activation(out=ysb, in_=yps, func=Act.Identity,
                                         bias=bias5[:, ec, :],
                                         accum_out=s_lin)
                    yabs = work2.tile([128, N], fp32, tag="yabs")
                    s_abs = work2.tile([128, 1], fp32, tag="s_abs")
                    nc.scalar.activation(out=yabs, in_=yps, func=Act.Abs,
                                         bias=bias5[:, ec, :],
                                         accum_out=s_abs)
                    # sum(lrelu(y)) = 0.6*sum(y) + 0.4*sum(|y|)
                    nc.vector.scalar_tensor_tensor(
                        out=gpool[:, ec, 1, cl:cl + 1], in0=s_abs,
                        scalar=(1.0 - SLOPE) / (1.0 + SLOPE), in1=s_lin,
                        op0=Alu.mult, op1=Alu.add)
                    nc.vector.tensor_scalar_mul(
                        gpool[:, ec, 1, cl:cl + 1],
                        gpool[:, ec, 1, cl:cl + 1], (1.0 + SLOPE) / 2.0)
                    # max(lrelu(y)) = lrelu(max(y))
                    pmax = work2.tile([128, 1], fp32, tag="pmax")
                    nc.vector.tensor_reduce(
                        out=pmax, in_=ysb,
                        axis=mybir.AxisListType.X, op=Alu.max)
                    nc.vector.scalar_tensor_tensor(
                        out=gpool[:, ec, 0, cl:cl + 1], in0=pmax,
                        scalar=SLOPE, in1=pmax, op0=Alu.mult, op1=Alu.max)

            # mean = sum / N
            nc.vector.tensor_scalar_mul(gpool[:, :, 1, :],
                                        gpool[:, :, 1, :], 1.0 / N)

            # FC tail. g chunks: gc in [0,8) = max emb chunks, [8,16) = mean
            z1 = work2.tile([128, 4, NCL], fp32, tag="z1")
            for zc in range(4):
                zps = psZ.tile([128, NCL], fp32, tag="zps")
                for gc in range(16):
                    which, ec = gc // 8, gc % 8
                    nc.tensor.matmul(
                        zps, lhsT=mmdt(l1[:, gc, ts(zc, 128)]),
                        rhs=mmdt(gpool[:, ec, which, :]),
                        start=(gc == 0), stop=(gc == 15))
                nc.scalar.activation(out=z1[:, zc, :], in_=zps,
                                     func=Act.Identity, bias=bias6[:, zc, :])
                nc.vector.scalar_tensor_tensor(
                    out=z1[:, zc, :], in0=z1[:, zc, :], scalar=SLOPE,
                    in1=z1[:, zc, :], op0=Alu.mult, op1=Alu.max)
            z2 = work2.tile([128, 2, NCL], fp32, tag="z2")
            for zc in range(2):
                zps = psZ.tile([128, NCL], fp32, tag="zps")
                for gc in range(4):
                    nc.tensor.matmul(
                        zps, lhsT=mmdt(l2[:, gc, ts(zc, 128)]),
                        rhs=mmdt(z1[:, gc, :]),
                        start=(gc == 0), stop=(gc == 3))
                nc.scalar.activation(out=z2[:, zc, :], in_=zps,
                                     func=Act.Identity, bias=bias7[:, zc, :])
                nc.vector.scalar_tensor_tensor(
                    out=z2[:, zc, :], in0=z2[:, zc, :], scalar=SLOPE,
                    in1=z2[:, zc, :], op0=Alu.mult, op1=Alu.max)
            zps = psZ.tile([40, NCL], fp32, tag="zps40")
            for gc in range(2):
                nc.tensor.matmul(zps, lhsT=mmdt(l3[:, gc, :]),
                                 rhs=mmdt(z2[:, gc, :]),
                                 start=(gc == 0), stop=(gc == 1))
            zf = work2.tile([40, NCL], fp32, tag="zf")
            nc.scalar.activation(out=zf, in_=zps, func=Act.Identity,
                                 bias=bias8)
            nc.sync.dma_start(out=z_out.rearrange("c o -> o c"), in_=zf)


def _edge_conv(tc, nc, mybir, P, xin, xout, ci, oi, wa, wb, bias, iota,
               ones_col, mmdt):
    fp32 = mybir.dt.float32
    fp16 = mybir.dt.float16
    i32 = mybir.dt.int32
    i16 = mybir.dt.int16
    Alu = mybir.AluOpType
    Act = mybir.ActivationFunctionType

    work, work1, pdpool, gath = P['work'], P['work1'], P['pdpool'], P['gath']
    psPD, psB, psA, dram = P['psPD'], P['psB'], P['psA'], P['dram']

    o_chunks = max(1, oi // 128)

    # ---- d, augmented rows ----
    xsq = work.tile([ci, N], fp32, tag="xsq")
    nc.scalar.activation(out=xsq, in_=xin, func=Act.Square)
    dps = psB.tile([1, N], fp32, tag="bigps")
    for nb in range(2):
        nc.tensor.matmul(dps[:, ts(nb, 512)], lhsT=ones_col[:ci, :],
                         rhs=xsq[:, ts(nb, 512)], start=True, stop=True)
    lhsT2x = work.tile([ci, N], fp32, tag="lhsT2x")
    nc.vector.tensor_scalar_mul(lhsT2x, xin, 2.0)
    # aug rows: engines can only write partition 0, so build single-row
    # tiles and DMA them into partition 1 of the aug tiles.
    d_row = work.tile([1, N], fp32, tag="d_row")
    nc.scalar.activation(out=d_row, in_=dps, func=Act.Copy)
    aug_l = work.tile([2, N], fp32, tag="aug_l")      # [ones; d]
    nc.vector.memset(aug_l[0:1, :], 1.0)
    nc.sync.dma_start(out=aug_l[1:2, :], in_=d_row)
    aug_r = work.tile([2, N], fp32, tag="aug_r")      # [-d; -ones]
    nc.scalar.activation(out=aug_r[0:1, :], in_=dps, func=Act.Copy,
                         scale=-1.0)
    nc.sync.dma_start(out=aug_r[1:2, :], in_=P['neg_ones'])

    # ---- A (O, N) fp32: same orientation as Bc ----
    a_sb = gath.tile([128, o_chunks * N], fp32, tag="a_sb")
    for oc in range(o_chunks):
        ow = min(oi, 128)
        for nb in range(2):
            aps = psA.tile([128, 512], fp32, tag="aps")
            nc.tensor.matmul(
                aps[:ow, :], lhsT=mmdt(wa[:, oc * 128:oc * 128 + ow]),
                rhs=mmdt(xin[:, ts(nb, 512)]), start=True, stop=True)
            nc.scalar.activation(
                out=a_sb[:ow, oc * N + nb * 512:oc * N + (nb + 1) * 512],
                in_=aps[:ow, :], func=Act.Copy)

    # ---- Bc (O, N) fp32 with folded bias ----
    bc_sb = work1.tile([128, o_chunks * N], fp32, tag="bc_sb")
    for oc in range(o_chunks):
        ow = min(oi, 128)
        bps = psB.tile([ow, N], fp32, tag="bigps")
        for nb in range(2):
            nc.tensor.matmul(
                bps[:, ts(nb, 512)],
                lhsT=mmdt(wb[:, oc * 128:oc * 128 + ow]),
                rhs=mmdt(xin[:, ts(nb, 512)]), start=True, stop=True)
        nc.scalar.activation(out=bc_sb[:ow, ts(oc, N)], in_=bps,
                             func=Act.Identity, bias=bias[:ow, oc, :])

    # ---- pd + topk ----
    idx16 = work.tile([128, 8, 24], i16, tag="idx16")
    for rc in range(8):
        pdps = psPD.tile([128, N], fp32, tag="pdps")
        for nb in range(2):
            nc.tensor.matmul(pdps[:, ts(nb, 512)],
                             lhsT=mmdt(lhsT2x[:, ts(rc, 128)]),
                             rhs=mmdt(xin[:, ts(nb, 512)]),
                             start=True, stop=False)
            nc.tensor.matmul(pdps[:, ts(nb, 512)],
                             lhsT=mmdt(aug_l[:, ts(rc, 128)]),
                             rhs=mmdt(aug_r[:, ts(nb, 512)]),
                             start=False, stop=True)
        pdsb = pdpool.tile([128, N], i32, tag="pdsb")
        nc.vector.scalar_tensor_tensor(
            out=pdsb, in0=pdps.bitcast(i32), scalar=P['mask_hi'], in1=iota,
            op0=Alu.bitwise_and, op1=Alu.bitwise_or)
        pdf = pdsb.bitcast(fp32)
        cand = pdpool.tile([128, 128], fp32, tag="cand")
        for s in range(16):
            nc.vector.max(out=cand[:, ts(s, 8)], in_=pdf[:, ts(s, 64)])
        top24 = pdpool.tile([128, 24], fp32, tag="top24")
        nc.vector.max(out=top24[:, 0:8], in_=cand)
        nc.vector.match_replace(out=cand, in_to_replace=top24[:, 0:8],
                                in_values=cand, imm_value=-1e30)
        nc.vector.max(out=top24[:, 8:16], in_=cand)
        nc.vector.match_replace(out=cand, in_to_replace=top24[:, 8:16],
                                in_values=cand, imm_value=-1e30)
        nc.vector.max(out=top24[:, 16:24], in_=cand)
        idx32 = pdpool.tile([128, 24], i32, tag="idx32")
        nc.vector.scalar_tensor_tensor(
            out=idx32, in0=top24.bitcast(i32), scalar=P['mask_lo'],
            in1=P['mask_lo'].to_broadcast([128, 24]),
            op0=Alu.bitwise_and, op1=Alu.bitwise_and)
        nc.vector.tensor_copy(
            out=idx16[:, rc, :],
            in_=idx32.bitcast(i16).rearrange("p (a b) -> p a b", b=2)[:, :, 0])

    # ---- gather + reduce + lrelu(M + Bc), per 128-row chunk ----
    # gather position within chunk rc: i = (nhi*20 + k)*16 + nlo;
    # wrap layout: p = i%16 = nlo, slot = i//16 = nhi*20 + k.
    # Full-image dram bounce: addr = nlo*1280 + (rc*8+nhi)*20 + k so the
    # per-rc wrapped image is dscratch[:, rc*160:(rc+1)*160].
    NIDX = N * KNN
    ow = min(oi, 128)
    dscratch = dram.tile([16, NIDX // 16], i16, tag="dscratch",
                         name="dscratch")
    dst = dscratch.rearrange("nlo (rc nhi k) -> nhi nlo rc k",
                             rc=8, nhi=8, k=KNN)
    nc.sync.dma_start(out=dst, in_=idx16[:, :, :KNN])
    idxw = work.tile([128, NIDX // 16], i16, tag="idxw", name="idxw")
    for grp in range(8):
        nc.sync.dma_start(out=idxw[grp * 16:(grp + 1) * 16, :], in_=dscratch)
    NR = 128 * KNN                            # 2560 idxs per row chunk
    for rc in range(8):
        for oc in range(o_chunks):
            gthr = gath.tile([128, NR], fp32, tag="gthr", name="gthr")
            cw = max(16, (ow + 15) // 16 * 16)
            nc.gpsimd.ap_gather(
                out_ap=gthr[:cw, :], in_ap=a_sb[:cw, ts(oc, N)],
                idxs_ap=idxw[:cw, ts(rc, NR // 16)],
                channels=cw,
                num_elems=N, d=1, num_idxs=NR)
            g4 = gthr[:ow, :].rearrange("p (nhi k nlo) -> p nhi nlo k",
                                        nhi=8, k=KNN, nlo=16)
            m32 = gath.tile([128, 128], fp32, tag="m32", name="m32")
            nc.vector.tensor_reduce(
                out=m32[:ow, :].rearrange("p (nhi nlo) -> p nhi nlo", nhi=8),
                in_=g4, axis=mybir.AxisListType.X, op=Alu.max)
            osl = slice(oc * N + rc * 128, oc * N + rc * 128 + 128)
            nc.vector.tensor_tensor(out=xout[:ow, osl], in0=m32[:ow, :],
                                    in1=bc_sb[:ow, osl], op=Alu.add)
            nc.vector.scalar_tensor_tensor(
                out=xout[:ow, osl], in0=xout[:ow, osl], scalar=SLOPE,
                in1=xout[:ow, osl], op0=Alu.mult, op1=Alu.max)


def kernel(x, params):
    from concourse.bass_utils import run_bass_kernel_spmd

    x = np.asarray(x, np.float32)
    pp = prep_params(params)
    if 'nc' not in _COMPILED:
        _COMPILED['nc'] = build_kernel()
    nc = _COMPILED['nc']

    in_maps = [make_in_map(x[core * NCL:(core + 1) * NCL], pp)
               for core in range(NCORES)]
    res = run_bass_kernel_spmd(nc, in_maps, core_ids=list(range(NCORES)))
    return np.concatenate([r['z'] for r in res.results],
                          axis=0).astype(np.float32)


# revision 27
# speedup vs baseline: 1.2726x; 1.0439x over previous
"""DGCNN classification forward pass on 8 Trainium2 NeuronCores.

Data-parallel over batch: B=16 point clouds, 2 per core. All tensor compute
runs on device; the host only shards inputs, folds BN params into weights
(constant prep), and concatenates outputs.

Per-cloud edge conv (C -> O), exploiting that BN scale>0 and LeakyReLU are
monotonic so max over neighbors commutes with the per-channel affine:
  out[o,n] = lrelu( max_k A[idx[n,k], o] + Bc[o,n] )
  A  = X^T (s*W_nbr)^T            (N, O)  -- "neighbor" projection
  Bc = (s*(W_ctr - W_nbr)) X + c  (O, N)  -- "center" part + folded BN bias

Steps:
  1. pd = 2 X^T X - d_i - d_j via PE matmul with augmented rows
     (lhsT = [2X; d; 1], rhs = [X; -1; -d]); output PSUM (128, 1024)/chunk.
  2. top-20 of each pd row: column index packed into the low 10 mantissa
     bits (one scalar_tensor_tensor AND/OR from PSUM), then top-8 per
     64-wide segment via vector.max, and top-24 of the 128 candidates via
     max/match_replace rounds. Low 10 bits of the winners = neighbor ids.
  3. gpsimd ap_gather on the (O, N) fp32 projection A gathers the 20
     neighbor columns per point (indices wrapped per 16 partitions).
  4. vector.tensor_reduce max over k -> M (O, N); out = lrelu(M + Bc).
Then conv5 (512->1024 pointwise) + global max/mean pool + 3 FC layers.
"""

import os
import numpy as np

B, C0, N = 16, 3, 1024
KNN = 20
EPS = 1e-5
SLOPE = 0.2
NCORES = 8
NCL = B // NCORES          # clouds per core
EMB = 1024

CONVS = [(3, 64), (64, 64), (64, 128), (128, 256)]   # (C_in, O)

_COMPILED = {}


def _fold_bn(bnp):
    g, b, m, v = [np.asarray(t, np.float64) for t in bnp]
    s = g / np.sqrt(v + EPS)
    c = b - s * m
    return s.astype(np.float32), c.astype(np.float32)


def prep_params(params):
    p = params
    out = {}
    for i, (ci, oi) in enumerate(CONVS, start=1):
        W = np.asarray(p[f'W{i}'], np.float32)
        s, c = _fold_bn(p[f'bn{i}'])
        Wn = (W[:, :ci] * s[:, None]).astype(np.float32)
        Wc = ((W[:, ci:] - W[:, :ci]) * s[:, None]).astype(np.float32)
        out[f'wa{i}'] = np.ascontiguousarray(Wn.T)               # (C, O)
        out[f'wb{i}'] = np.ascontiguousarray(Wc.T)               # (C, O)
        out[f'bias{i}'] = np.ascontiguousarray(
            c.reshape(-1, 128, 1) if oi > 128 else c.reshape(1, oi, 1))
    s5, c5 = _fold_bn(p['bn5'])
    W5 = np.asarray(p['W5'], np.float32) * s5[:, None]           # (1024, 512)
    W5T = np.ascontiguousarray(W5.T)                             # (512, 1024)
    # contraction pieces matching h = [x1(64); x2(64); x3(128); x4(256)]
    for k, (r0, r1) in enumerate([(0, 64), (64, 128), (128, 256),
                                  (256, 384), (384, 512)]):
        out[f'w5p{k}'] = np.ascontiguousarray(W5T[r0:r1])
    out['bias5'] = np.ascontiguousarray(c5.reshape(8, 128).T[:, :, None])
    s6, c6 = _fold_bn(p['bn6'])
    L1 = np.asarray(p['L1'], np.float32) * s6[:, None]           # (512, 2048)
    out['l1T'] = np.ascontiguousarray(
        L1.T.reshape(16, 128, 512).transpose(1, 0, 2))           # (128, 16, 512)
    out['bias6'] = np.ascontiguousarray(c6.reshape(4, 128).T[:, :, None])
    s7, c7 = _fold_bn(p['bn7'])
    L2 = np.asarray(p['L2'], np.float32) * s7[:, None]           # (256, 512)
    out['l2T'] = np.ascontiguousarray(
        L2.T.reshape(4, 128, 256).transpose(1, 0, 2))            # (128, 4, 256)
    out['bias7'] = np.ascontiguousarray(
        (s7 * np.asarray(p['b2'], np.float32) + c7)
        .reshape(2, 128).T[:, :, None])                          # (128, 2, 1)
    L3 = np.asarray(p['L3'], np.float32)                         # (40, 256)
    out['l3T'] = np.ascontiguousarray(
        L3.T.reshape(2, 128, 40).transpose(1, 0, 2))             # (128, 2, 40)
    out['bias8'] = np.asarray(p['b3'], np.float32)[:, None]      # (40, 1)
    return out


def make_in_map(x_shard, pp):
    m = {'x0': np.ascontiguousarray(x_shard, dtype=np.float32),
         'iota': np.broadcast_to(np.arange(N, dtype=np.int32)[None, :],
                                 (128, N)).copy()}
    for k, v in pp.items():
        m[k] = v
    return m


def ts(i, size):
    return slice(i * size, (i + 1) * size)


def build_kernel():
    import concourse.bass as bass
    import concourse.mybir as mybir
    from concourse import bacc
    from concourse.tile import TileContext

    fp32 = mybir.dt.float32
    i32 = mybir.dt.int32

    nc = bacc.Bacc("TRN2", debug=False)

    ins = {}
    ins['x0'] = nc.dram_tensor("x0", [NCL, C0, N], fp32,
                               kind="ExternalInput").ap()
    ins['iota'] = nc.dram_tensor("iota", [128, N], i32,
                                 kind="ExternalInput").ap()
    for i, (ci, oi) in enumerate(CONVS, start=1):
        ins[f'wa{i}'] = nc.dram_tensor(f"wa{i}", [ci, oi], fp32,
                                       kind="ExternalInput").ap()
        ins[f'wb{i}'] = nc.dram_tensor(f"wb{i}", [ci, oi], fp32,
                                       kind="ExternalInput").ap()
        ins[f'bias{i}'] = nc.dram_tensor(
            f"bias{i}", [max(1, oi // 128), min(oi, 128), 1], fp32,
            kind="ExternalInput").ap()
    for k, kw in enumerate([64, 64, 128, 128, 128]):
        ins[f'w5p{k}'] = nc.dram_tensor(f"w5p{k}", [kw, EMB], fp32,
                                        kind="ExternalInput").ap()
    ins['bias5'] = nc.dram_tensor("bias5", [128, 8, 1], fp32,
                                  kind="ExternalInput").ap()
    ins['l1T'] = nc.dram_tensor("l1T", [128, 16, 512], fp32,
                                kind="ExternalInput").ap()
    ins['bias6'] = nc.dram_tensor("bias6", [128, 4, 1], fp32,
                                  kind="ExternalInput").ap()
    ins['l2T'] = nc.dram_tensor("l2T", [128, 4, 256], fp32,
                                kind="ExternalInput").ap()
    ins['bias7'] = nc.dram_tensor("bias7", [128, 2, 1], fp32,
                                  kind="ExternalInput").ap()
    ins['l3T'] = nc.dram_tensor("l3T", [128, 2, 40], fp32,
                                kind="ExternalInput").ap()
    ins['bias8'] = nc.dram_tensor("bias8", [40, 1], fp32,
                                  kind="ExternalInput").ap()
    z_out = nc.dram_tensor("z", [NCL, 40], fp32, kind="ExternalOutput").ap()

    with TileContext(nc) as tc:
        _emit(tc, nc, mybir, bass, ins, z_out)
    nc.compile()
    return nc


def _emit(tc, nc, mybir, bass, ins, z_out):
    from contextlib import ExitStack
    fp32 = mybir.dt.float32
    fp32r = mybir.dt.float32r
    fp16 = mybir.dt.float16
    i32 = mybir.dt.int32
    i16 = mybir.dt.int16
    Alu = mybir.AluOpType
    Act = mybir.ActivationFunctionType
    MM_DT = os.environ.get("DGCNN_MM_DT", "fp32")

    def mmdt(ap):
        return ap.bitcast(fp32r) if MM_DT == "fp32r" else ap

    with ExitStack() as top:
        persist = top.enter_context(tc.tile_pool(name="persist", bufs=1))

        # persistent per-cloud features: x0 and conv outputs (chunk-in-free
        # for O=256). All fp32, layout (min(C,128), n_chunks*N).
        feats = {}
        for cl in range(NCL):
            feats[(cl, 0)] = persist.tile([C0, N], fp32, tag=f"x0_{cl}", name=f"x0_{cl}")
            for i, (ci, oi) in enumerate(CONVS, start=1):
                feats[(cl, i)] = persist.tile(
                    [min(oi, 128), max(1, oi // 128) * N], fp32,
                    tag=f"x{i}_{cl}", name=f"x{i}_{cl}")
        # global pool accumulators: [128, emb_chunk 8, {max,sum} 2, cl]
        gpool = persist.tile([128, 8, 2, NCL], fp32)

        # ---------------- phase 1: edge convs ----------------
        with ExitStack() as ph1:
            const = ph1.enter_context(tc.tile_pool(name="const1", bufs=1))
            work = ph1.enter_context(tc.tile_pool(name="work", bufs=2))
            work1 = ph1.enter_context(tc.tile_pool(name="work1", bufs=1))
            pdpool = ph1.enter_context(tc.tile_pool(name="pdpool", bufs=3))
            gath = ph1.enter_context(tc.tile_pool(name="gath", bufs=2))
            dram = ph1.enter_context(
                tc.tile_pool(name="dram", bufs=2, space="DRAM"))
            psPD = ph1.enter_context(
                tc.tile_pool(name="psPD", bufs=2, space="PSUM"))
            psB = ph1.enter_context(
                tc.tile_pool(name="psB", bufs=1, space="PSUM"))
            psA = ph1.enter_context(
                tc.tile_pool(name="psA", bufs=2, space="PSUM"))

            from concourse import library_config
            nc.gpsimd.load_library(library_config.mlp)
            iota = const.tile([128, N], i32)
            nc.sync.dma_start(out=iota, in_=ins['iota'])
            ones_col = const.tile([128, 1], fp32)
            nc.vector.memset(ones_col, 1.0)
            neg_ones = const.tile([1, N], fp32)
            nc.vector.memset(neg_ones, -1.0)
            mask_hi = const.tile([128, 1], i32)
            nc.vector.memset(mask_hi, -1024)
            mask_lo = const.tile([128, 1], i32)
            nc.vector.memset(mask_lo, 1023)
            wa_sb, wb_sb, bias_sb = {}, {}, {}
            for i, (ci, oi) in enumerate(CONVS, start=1):
                wa_sb[i] = const.tile([ci, oi], fp32, tag=f"wa{i}", name=f"wa{i}_sb")
                nc.sync.dma_start(out=wa_sb[i], in_=ins[f'wa{i}'])
                wb_sb[i] = const.tile([ci, oi], fp32, tag=f"wb{i}", name=f"wb{i}_sb")
                nc.sync.dma_start(out=wb_sb[i], in_=ins[f'wb{i}'])
                bias_sb[i] = const.tile([min(oi, 128), max(1, oi // 128), 1],
                                        fp32, tag=f"bias{i}", name=f"bias{i}_sb")
                nc.sync.dma_start(
                    out=bias_sb[i],
                    in_=ins[f'bias{i}'].rearrange("a b c -> b a c"))

            for cl in range(NCL):
                nc.sync.dma_start(out=feats[(cl, 0)], in_=ins['x0'][cl])

            P = dict(const=const, work=work, work1=work1, pdpool=pdpool,
                     gath=gath, dram=dram, psPD=psPD, psB=psB, psA=psA,
                     neg_ones=neg_ones, mask_hi=mask_hi, mask_lo=mask_lo)
            for cl in range(NCL):
                for i, (ci, oi) in enumerate(CONVS, start=1):
                    _edge_conv(tc, nc, mybir, P,
                               xin=feats[(cl, i - 1)],
                               xout=feats[(cl, i)],
                               ci=ci, oi=oi, wa=wa_sb[i], wb=wb_sb[i],
                               bias=bias_sb[i], iota=iota,
                               ones_col=ones_col, mmdt=mmdt)

        # ---------------- phase 2: conv5 + pool + FC tail ----------------
        with ExitStack() as ph2:
            const2 = ph2.enter_context(tc.tile_pool(name="const2", bufs=1))
            work2 = ph2.enter_context(tc.tile_pool(name="work2", bufs=2))
            psY = ph2.enter_context(
                tc.tile_pool(name="psY", bufs=2, space="PSUM"))
            psZ = ph2.enter_context(
                tc.tile_pool(name="psZ", bufs=2, space="PSUM"))

            w5p = []
            for k, kw in enumerate([64, 64, 128, 128, 128]):
                t = const2.tile([kw, EMB], fp32, tag=f"w5p{k}", name=f"w5p{k}_sb")
                nc.sync.dma_start(out=t, in_=ins[f'w5p{k}'])
                w5p.append(t)
            bias5 = const2.tile([128, 8, 1], fp32)
            nc.sync.dma_start(out=bias5, in_=ins['bias5'])
            l1 = const2.tile([128, 16, 512], fp32)
            nc.sync.dma_start(out=l1, in_=ins['l1T'])
            bias6 = const2.tile([128, 4, 1], fp32)
            nc.sync.dma_start(out=bias6, in_=ins['bias6'])
            l2 = const2.tile([128, 4, 256], fp32)
            nc.sync.dma_start(out=l2, in_=ins['l2T'])
            bias7 = const2.tile([128, 2, 1], fp32)
            nc.sync.dma_start(out=bias7, in_=ins['bias7'])
            l3 = const2.tile([128, 2, 40], fp32)
            nc.sync.dma_start(out=l3, in_=ins['l3T'])
            bias8 = const2.tile([40, 1], fp32)
            nc.sync.dma_start(out=bias8, in_=ins['bias8'])

            for cl in range(NCL):
                # h = [x1; x2; x3; x4]: contraction pieces (tile, chunk, rows)
                x1, x2, x3, x4 = (feats[(cl, i)] for i in (1, 2, 3, 4))
                pieces = [(x1, 0, 64), (x2, 0, 64), (x3, 0, 128),
                          (x4, 0, 128), (x4, 1, 128)]
                for ec in range(8):
                    yps = psY.tile([128, N], fp32, tag="yps")
                    for nb in range(2):
                        for ki, (xt, xc, kw) in enumerate(pieces):
                            nc.tensor.matmul(
                                yps[:, ts(nb, 512)],
                                lhsT=mmdt(w5p[ki][:, ts(ec, 128)]),
                                rhs=mmdt(xt[:kw,
                                            xc * N + nb * 512:
                                            xc * N + (nb + 1) * 512]),
                                start=(ki == 0), stop=(ki == len(pieces) - 1))
                    ysb = work2.tile([128, N], fp32, tag="ysb")
                    s_lin = work2.tile([128, 1], fp32, tag="s_lin")
                    nc.scalar.activation(out=ysb, in_=yps, func=Act.Identity,
                                         bias=bias5[:, ec, :],
                                         accum_out=s_lin)
                    yabs = work2.tile([128, N], fp32, tag="yabs")
                    s_abs = work2.tile([128, 1], fp32, tag="s_abs")
                    nc.scalar.activation(out=yabs, in_=yps, func=Act.Abs,
                                         bias=bias5[:, ec, :],
                                         accum_out=s_abs)
                    # sum(lrelu(y)) = 0.6*sum(y) + 0.4*sum(|y|)
                    nc.vector.scalar_tensor_tensor(
                        out=gpool[:, ec, 1, cl:cl + 1], in0=s_abs,
                        scalar=(1.0 - SLOPE) / (1.0 + SLOPE), in1=s_lin,
                        op0=Alu.mult, op1=Alu.add)
                    nc.vector.tensor_scalar_mul(
                        gpool[:, ec, 1, cl:cl + 1],
                        gpool[:, ec, 1, cl:cl + 1], (1.0 + SLOPE) / 2.0)
                    # max(lrelu(y)) = lrelu(max(y))
                    pmax = work2.tile([128, 1], fp32, tag="pmax")
                    nc.vector.tensor_reduce(
                        out=pmax, in_=ysb,
                        axis=mybir.AxisListType.X, op=Alu.max)
                    nc.vector.scalar_tensor_tensor(
                        out=gpool[:, ec, 0, cl:cl + 1], in0=pmax,
                        scalar=SLOPE, in1=pmax, op0=Alu.mult, op1=Alu.max)

            # mean = sum / N
            nc.vector.tensor_scalar_mul(gpool[:, :, 1, :],
                                        gpool[:, :, 1, :], 1.0 / N)

            # FC tail. g chunks: gc in [0,8) = max emb chunks, [8,16) = mean
            z1 = work2.tile([128, 4, NCL], fp32, tag="z1")
            for zc in range(4):
                zps = psZ.tile([128, NCL], fp32, tag="zps")
                for gc in range(16):
                    which, ec = gc // 8, gc % 8
                    nc.tensor.matmul(
                        zps, lhsT=mmdt(l1[:, gc, ts(zc, 128)]),
                        rhs=mmdt(gpool[:, ec, which, :]),
                        start=(gc == 0), stop=(gc == 15))
                nc.scalar.activation(out=z1[:, zc, :], in_=zps,
                                     func=Act.Identity, bias=bias6[:, zc, :])
                nc.vector.scalar_tensor_tensor(
                    out=z1[:, zc, :], in0=z1[:, zc, :], scalar=SLOPE,
                    in1=z1[:, zc, :], op0=Alu.mult, op1=Alu.max)
            z2 = work2.tile([128, 2, NCL], fp32, tag="z2")
            for zc in range(2):
                zps = psZ.tile([128, NCL], fp32, tag="zps")
                for gc in range(4):
                    nc.tensor.matmul(
                        zps, lhsT=mmdt(l2[:, gc, ts(zc, 128)]),
                        rhs=mmdt(z1[:, gc, :]),
                        start=(gc == 0), stop=(gc == 3))
                nc.scalar.activation(out=z2[:, zc, :], in_=zps,
                                     func=Act.Identity, bias=bias7[:, zc, :])
                nc.vector.scalar_tensor_tensor(
                    out=z2[:, zc, :], in0=z2[:, zc, :], scalar=SLOPE,
                    in1=z2[:, zc, :], op0=Alu.mult, op1=Alu.max)
            zps = psZ.tile([40, NCL], fp32, tag="zps40")
            for gc in range(2):
                nc.tensor.matmul(zps, lhsT=mmdt(l3[:, gc, :]),
                                 rhs=mmdt(z2[:, gc, :]),
                                 start=(gc == 0), stop=(gc == 1))
            zf = work2.tile([40, NCL], fp32, tag="zf")
            nc.scalar.activation(out=zf, in_=zps, func=Act.Identity,
                                 bias=bias8)
            nc.sync.dma_start(out=z_out.rearrange("c o -> o c"), in_=zf)


def _edge_conv(tc, nc, mybir, P, xin, xout, ci, oi, wa, wb, bias, iota,
               ones_col, mmdt):
    fp32 = mybir.dt.float32
    fp16 = mybir.dt.float16
    i32 = mybir.dt.int32
    i16 = mybir.dt.int16
    Alu = mybir.AluOpType
    Act = mybir.ActivationFunctionType

    work, work1, pdpool, gath = P['work'], P['work1'], P['pdpool'], P['gath']
    psPD, psB, psA, dram = P['psPD'], P['psB'], P['psA'], P['dram']

    o_chunks = max(1, oi // 128)

    # ---- d, augmented rows ----
    xsq = work.tile([ci, N], fp32, tag="xsq")
    nc.scalar.activation(out=xsq, in_=xin, func=Act.Square)
    dps = psB.tile([1, N], fp32, tag="bigps")
    for nb in range(2):
        nc.tensor.matmul(dps[:, ts(nb, 512)], lhsT=ones_col[:ci, :],
                         rhs=xsq[:, ts(nb, 512)], start=True, stop=True)
    lhsT2x = work.tile([ci, N], fp32, tag="lhsT2x")
    nc.vector.tensor_scalar_mul(lhsT2x, xin, 2.0)
    # aug rows: engines can only write partition 0, so build single-row
    # tiles and DMA them into partition 1 of the aug tiles.
    d_row = work.tile([1, N], fp32, tag="d_row")
    nc.scalar.activation(out=d_row, in_=dps, func=Act.Copy)
    aug_l = work.tile([2, N], fp32, tag="aug_l")      # [ones; d]
    nc.vector.memset(aug_l[0:1, :], 1.0)
    nc.sync.dma_start(out=aug_l[1:2, :], in_=d_row)
    aug_r = work.tile([2, N], fp32, tag="aug_r")      # [-d; -ones]
    nc.scalar.activation(out=aug_r[0:1, :], in_=dps, func=Act.Copy,
                         scale=-1.0)
    nc.sync.dma_start(out=aug_r[1:2, :], in_=P['neg_ones'])

    # ---- A (O, N) fp32: same orientation as Bc ----
    a_sb = gath.tile([128, o_chunks * N], fp32, tag="a_sb")
    for oc in range(o_chunks):
        ow = min(oi, 128)
        for nb in range(2):
            aps = psA.tile([128, 512], fp32, tag="aps")
            nc.tensor.matmul(
                aps[:ow, :], lhsT=mmdt(wa[:, oc * 128:oc * 128 + ow]),
                rhs=mmdt(xin[:, ts(nb, 512)]), start=True, stop=True)
            nc.scalar.activation(
                out=a_sb[:ow, oc * N + nb * 512:oc * N + (nb + 1) * 512],
                in_=aps[:ow, :], func=Act.Copy)

    # ---- Bc (O, N) fp32 with folded bias ----
    bc_sb = work1.tile([128, o_chunks * N], fp32, tag="bc_sb")
    for oc in range(o_chunks):
        ow = min(oi, 128)
        bps = psB.tile([ow, N], fp32, tag="bigps")
        for nb in range(2):
            nc.tensor.matmul(
                bps[:, ts(nb, 512)],
                lhsT=mmdt(wb[:, oc * 128:oc * 128 + ow]),
                rhs=mmdt(xin[:, ts(nb, 512)]), start=True, stop=True)
        nc.scalar.activation(out=bc_sb[:ow, ts(oc, N)], in_=bps,
                             func=Act.Identity, bias=bias[:ow, oc, :])

    # ---- pd + topk ----
    idx16 = work.tile([128, 8, 24], i16, tag="idx16")
    for rc in range(8):
        pdps = psPD.tile([128, N], fp32, tag="pdps")
        for nb in range(2):
            nc.tensor.matmul(pdps[:, ts(nb, 512)],
                             lhsT=mmdt(lhsT2x[:, ts(rc, 128)]),
                             rhs=mmdt(xin[:, ts(nb, 512)]),
                             start=True, stop=False)
            nc.tensor.matmul(pdps[:, ts(nb, 512)],
                             lhsT=mmdt(aug_l[:, ts(rc, 128)]),
                             rhs=mmdt(aug_r[:, ts(nb, 512)]),
                             start=False, stop=True)
        pdsb = pdpool.tile([128, N], i32, tag="pdsb")
        nc.vector.scalar_tensor_tensor(
            out=pdsb, in0=pdps.bitcast(i32), scalar=P['mask_hi'], in1=iota,
            op0=Alu.bitwise_and, op1=Alu.bitwise_or)
        pdf = pdsb.bitcast(fp32)
        cand = pdpool.tile([128, 64], fp32, tag="cand")
        for s in range(8):
            nc.vector.max(out=cand[:, ts(s, 8)], in_=pdf[:, ts(s, 128)])
        top24 = pdpool.tile([128, 24], fp32, tag="top24")
        nc.vector.max(out=top24[:, 0:8], in_=cand)
        nc.vector.match_replace(out=cand, in_to_replace=top24[:, 0:8],
                                in_values=cand, imm_value=-1e30)
        nc.vector.max(out=top24[:, 8:16], in_=cand)
        nc.vector.match_replace(out=cand, in_to_replace=top24[:, 8:16],
                                in_values=cand, imm_value=-1e30)
        nc.vector.max(out=top24[:, 16:24], in_=cand)
        idx32 = pdpool.tile([128, 24], i32, tag="idx32")
        nc.vector.scalar_tensor_tensor(
            out=idx32, in0=top24.bitcast(i32), scalar=P['mask_lo'],
            in1=P['mask_lo'].to_broadcast([128, 24]),
            op0=Alu.bitwise_and, op1=Alu.bitwise_and)
        nc.vector.tensor_copy(
            out=idx16[:, rc, :],
            in_=idx32.bitcast(i16).rearrange("p (a b) -> p a b", b=2)[:, :, 0])

    # ---- gather + reduce + lrelu(M + Bc), per 128-row chunk ----
    # gather position within chunk rc: i = (nhi*20 + k)*16 + nlo;
    # wrap layout: p = i%16 = nlo, slot = i//16 = nhi*20 + k.
    # Full-image dram bounce: addr = nlo*1280 + (rc*8+nhi)*20 + k so the
    # per-rc wrapped image is dscratch[:, rc*160:(rc+1)*160].
    NIDX = N * KNN
    ow = min(oi, 128)
    dscratch = dram.tile([16, NIDX // 16], i16, tag="dscratch",
                         name="dscratch")
    dst = dscratch.rearrange("nlo (rc nhi k) -> nhi nlo rc k",
                             rc=8, nhi=8, k=KNN)
    nc.sync.dma_start(out=dst, in_=idx16[:, :, :KNN])
    idxw = work.tile([128, NIDX // 16], i16, tag="idxw", name="idxw")
    for grp in range(8):
        nc.sync.dma_start(out=idxw[grp * 16:(grp + 1) * 16, :], in_=dscratch)
    NR = 128 * KNN                            # 2560 idxs per row chunk
    for rc in range(8):
        for oc in range(o_chunks):
            gthr = gath.tile([128, NR], fp32, tag="gthr", name="gthr")
            cw = max(16, (ow + 15) // 16 * 16)
            nc.gpsimd.ap_gather(
                out_ap=gthr[:cw, :], in_ap=a_sb[:cw, ts(oc, N)],
                idxs_ap=idxw[:cw, ts(rc, NR // 16)],
                channels=cw,
                num_elems=N, d=1, num_idxs=NR)
            g4 = gthr[:ow, :].rearrange("p (nhi k nlo) -> p nhi nlo k",
                                        nhi=8, k=KNN, nlo=16)
            m32 = gath.tile([128, 128], fp32, tag="m32", name="m32")
            nc.vector.tensor_reduce(
                out=m32[:ow, :].rearrange("p (nhi nlo) -> p nhi nlo", nhi=8),
                in_=g4, axis=mybir.AxisListType.X, op=Alu.max)
            osl = slice(oc * N + rc * 128, oc * N + rc * 128 + 128)
            nc.vector.tensor_tensor(out=xout[:ow, osl], in0=m32[:ow, :],
                                    in1=bc_sb[:ow, osl], op=Alu.add)
            nc.vector.scalar_tensor_tensor(
                out=xout[:ow, osl], in0=xout[:ow, osl], scalar=SLOPE,
                in1=xout[:ow, osl], op0=Alu.mult, op1=Alu.max)


def kernel(x, params):
    from concourse.bass_utils import run_bass_kernel_spmd

    x = np.asarray(x, np.float32)
    pp = prep_params(params)
    if 'nc' not in _COMPILED:
        _COMPILED['nc'] = build_kernel()
    nc = _COMPILED['nc']

    in_maps = [make_in_map(x[core * NCL:(core + 1) * NCL], pp)
               for core in range(NCORES)]
    res = run_bass_kernel_spmd(nc, in_maps, core_ids=list(range(NCORES)))
    return np.concatenate([r['z'] for r in res.results],
                          axis=0).astype(np.float32)
